# revision 11
# baseline (speedup 1.0000x reference)
"""Contrastive loss (SimCLR-style semi_loss pair) on 8 Trainium2 NeuronCores.

Math (reference):
    z1n, z2n = L2-normalized rows of z1, z2            # [N, D], N=16384, D=128
    S11 = z1n @ z1n.T, S12 = z1n @ z2n.T, S22 = z2n @ z2n.T, S21 = S12.T
    d1_i = sum_j exp(2*S11_ij) - exp(2*S11_ii) + sum_j exp(2*S12_ij)
    d2_i = sum_j exp(2*S22_ij) - exp(2*S22_ii) + sum_j exp(2*S21_ij)
    loss = mean_i( 0.5*(log d1_i + log d2_i) - 2*S12_ii )

Sharding: row-parallel over N. Core c owns rows [c*2048, (c+1)*2048) and
computes its row-block of all four exp-Gram row-sums against the full
(replicated, feature-major) z1n/z2n. Host combines per-core partial
results (the "all-reduce" of the final mean happens in the host gather).

Device layout: D=128 features sit on the SBUF partition axis, so every
Gram tile is a single K=128 matmul: out[m, n] = sum_d blkT[d, m] * fullT[d, n].
ScalarE applies exp(2*x) in-place on 2048-wide PSUM spans (4 banks) with
accum_out producing the row-sum partials. Diagonals come from a separate
elementwise-product + ones-matmul column-sum pass (raw dot products).
"""

import os

import numpy as np

N = 16384
D = 128
NCORES = 8
B = N // NCORES  # 2048 rows per core
TAU = 0.5
SCALE = 1.0 / TAU
EPS = 1e-12

M_CH = B // 128  # 16 row chunks of 128 per core
SPAN = 2048  # ACT read span = 4 PSUM banks
NSP = N // SPAN  # 8 spans
KPS = SPAN // 512  # 4 matmuls per span

_cache = {}


def _build():
    """Build the Bass program (shared SPMD NEFF for all 8 cores)."""
    from contextlib import ExitStack

    import concourse.mybir as mybir
    from concourse import bacc
    from concourse.tile import TileContext

    f32 = mybir.dt.float32
    bf16 = mybir.dt.bfloat16
    Exp = mybir.ActivationFunctionType.Exp
    add = mybir.AluOpType.add
    AX = mybir.AxisListType.X

    # Bacc (vs plain Bass) runs the wait-legalization passes at finalize:
    # move_matmul_waits_to_ldweights + generate_event_semaphores (TRN2 allows
    # at most one sync wait per hardware instruction).
    nc = bacc.Bacc(None, target_bir_lowering=False, name="contrastive_loss")

    z1t = nc.declare_dram_parameter("z1t", [D, N], bf16, isOutput=False)
    z2t = nc.declare_dram_parameter("z2t", [D, N], bf16, isOutput=False)
    # block rows of z1 and z2, concatenated: one DMA -> one semaphore, so
    # downstream DVE tensor_mul needs only a single sync wait (the core_v3
    # TensorTensor struct has one wait slot)
    zb = nc.declare_dram_parameter("zb", [D, 2 * B], bf16, isOutput=False)

    rs_names = ["rs11", "rs12", "rs21", "rs22"]
    rs_dram = {
        s: nc.declare_dram_parameter(s, [128, M_CH], f32, isOutput=True)
        for s in rs_names
    }
    # raw dot-product diagonals: [d11 | d12 | d22], each B long
    dg_dram = nc.declare_dram_parameter("diags", [1, 3 * B], f32, isOutput=True)

    with TileContext(nc) as tc, ExitStack() as ctx:
        const = ctx.enter_context(tc.tile_pool(name="const", bufs=1))
        psum = ctx.enter_context(tc.tile_pool(name="psum", bufs=2, space="PSUM"))
        work = ctx.enter_context(tc.tile_pool(name="work", bufs=4))
        prodp = ctx.enter_context(tc.tile_pool(name="prodp", bufs=3))
        outp = ctx.enter_context(tc.tile_pool(name="outp", bufs=1))

        z1t_sb = const.tile([128, N], bf16)
        z2t_sb = const.tile([128, N], bf16)
        zb_sb = const.tile([128, 2 * B], bf16)
        # split the big loads across several DMA queues
        ncol = N // 4
        for i in range(4):
            nc.sync.dma_start(
                out=z1t_sb[:, i * ncol : (i + 1) * ncol],
                in_=z1t[:, i * ncol : (i + 1) * ncol],
            )
            nc.sync.dma_start(
                out=z2t_sb[:, i * ncol : (i + 1) * ncol],
                in_=z2t[:, i * ncol : (i + 1) * ncol],
            )
        nc.sync.dma_start(out=zb_sb, in_=zb[:, :])
        z1b_sb = zb_sb[:, 0:B]
        z2b_sb = zb_sb[:, B : 2 * B]

        ones = const.tile([128, 1], f32)
        nc.vector.memset(ones, 1.0)

        # ---- Phase 0: raw diagonals  diag[i] = sum_d a[d,i]*b[d,i] ----
        dg_sb = outp.tile([1, 3 * B], f32)
        pairs = [(z1b_sb, z1b_sb), (z1b_sb, z2b_sb), (z2b_sb, z2b_sb)]
        for di, (a, b) in enumerate(pairs):
            prod = prodp.tile([128, B], f32)
            nc.vector.tensor_mul(prod, a, b)
            ps = psum.tile([128, SPAN], f32, tag="sim")
            for k in range(KPS):
                nc.tensor.matmul(
                    ps[0:1, k * 512 : (k + 1) * 512],
                    lhsT=ones,
                    rhs=prod[:, k * 512 : (k + 1) * 512],
                    start=True,
                    stop=True,
                )
            nc.vector.tensor_copy(
                out=dg_sb[0:1, di * B : (di + 1) * B], in_=ps[0:1, 0:B]
            )
        nc.sync.dma_start(out=dg_dram[:, :], in_=dg_sb)

        # ---- Main streams: exp-Gram row sums ----
        streams = [
            ("rs11", z1b_sb, z1t_sb),
            ("rs12", z1b_sb, z2t_sb),
            ("rs21", z2b_sb, z1t_sb),
            ("rs22", z2b_sb, z2t_sb),
        ]
        for sname, blk, full in streams:
            rs_sb = outp.tile([128, M_CH], f32, tag=f"rs_{sname}")
            for m in range(M_CH):
                parts = work.tile([128, NSP], f32, tag="parts")
                for s in range(NSP):
                    sim = psum.tile([128, SPAN], f32, tag="sim")
                    for k in range(KPS):
                        col = s * SPAN + k * 512
                        nc.tensor.matmul(
                            sim[:, k * 512 : (k + 1) * 512],
                            lhsT=blk[:, m * 128 : (m + 1) * 128],
                            rhs=full[:, col : col + 512],
                            start=True,
                            stop=True,
                        )
                    nc.scalar.activation(
                        out=sim,
                        in_=sim,
                        func=Exp,
                        scale=SCALE,
                        accum_out=parts[:, s : s + 1],
                    )
                nc.vector.tensor_reduce(
                    out=rs_sb[:, m : m + 1], in_=parts, axis=AX, op=add
                )
            nc.sync.dma_start(out=rs_dram[sname][:, :], in_=rs_sb)

    nc.finalize()  # Bacc: runs wait-legalization + register allocation
    return nc


def _get_nc():
    if "nc" not in _cache:
        _cache["nc"] = _build()
    return _cache["nc"]


def kernel(z1: np.ndarray, z2: np.ndarray) -> np.ndarray:
    import ml_dtypes

    from concourse.bass_utils import run_bass_kernel_spmd

    z1 = np.asarray(z1, dtype=np.float32)
    z2 = np.asarray(z2, dtype=np.float32)

    # host: L2 row-normalize (matches F.normalize eps clamp), transpose to
    # feature-major, cast bf16
    def prep(z):
        n = np.sqrt((z.astype(np.float64) ** 2).sum(axis=1, keepdims=True))
        zn = (z / np.maximum(n, EPS).astype(np.float32)).astype(np.float32)
        return np.ascontiguousarray(zn.T).astype(ml_dtypes.bfloat16)

    z1tn = prep(z1)  # [D, N] bf16
    z2tn = prep(z2)

    core_ids = list(range(NCORES))
    in_maps = []
    for c in core_ids:
        sl = slice(c * B, (c + 1) * B)
        in_maps.append(
            {
                "z1t": z1tn,
                "z2t": z2tn,
                "zb": np.ascontiguousarray(
                    np.concatenate([z1tn[:, sl], z2tn[:, sl]], axis=1)
                ),
            }
        )

    nc = _get_nc()
    res = run_bass_kernel_spmd(
        nc,
        in_maps,
        core_ids,
        trace=bool(int(os.environ.get("KERNEL_TRACE", "0"))),
    )
    _cache["last_result"] = res

    # ---- host combine (the final all-reduce / mean) ----
    loss_sum = 0.0
    for c in core_ids:
        r = res.results[c]
        # rs tiles are [128, M_CH]: row (m*128 + p) -> [p, m]
        rs = {s: r[s].astype(np.float64).T.reshape(B) for s in ["rs11", "rs12", "rs21", "rs22"]}
        dg = r["diags"].astype(np.float64).reshape(3 * B)
        d11, d12, d22 = dg[0:B], dg[B : 2 * B], dg[2 * B : 3 * B]
        den1 = rs["rs11"] - np.exp(SCALE * d11) + rs["rs12"]
        den2 = rs["rs22"] - np.exp(SCALE * d22) + rs["rs21"]
        l = 0.5 * (np.log(den1) + np.log(den2)) - SCALE * d12
        loss_sum += l.sum()

    return np.float32(loss_sum / N)


# revision 20
# speedup vs baseline: 1.2696x; 1.2696x over previous
"""Contrastive loss (SimCLR-style semi_loss pair) on 8 Trainium2 NeuronCores.

Math (reference):
    z1n, z2n = L2-normalized rows of z1, z2            # [N, D], N=16384, D=128
    S11 = z1n @ z1n.T, S12 = z1n @ z2n.T, S22 = z2n @ z2n.T, S21 = S12.T
    d1_i = sum_j exp(2*S11_ij) - exp(2*S11_ii) + sum_j exp(2*S12_ij)
    d2_i = sum_j exp(2*S22_ij) - exp(2*S22_ii) + sum_j exp(2*S21_ij)
    loss = mean_i( 0.5*(log d1_i + log d2_i) - 2*S12_ii )

Sharding: row-parallel over N. Core c owns rows [c*2048, (c+1)*2048) and
computes its row-block of the exp-Gram row-sums against the full
(replicated, feature-major) z1n/z2n. exp(S21)-row-sums are column-sums of
exp(S12), so the S12 exponentials are computed once (ScalarE), written to
SBUF as bf16, and column-reduced with ones-vector matmuls that accumulate
in PSUM. Host combines per-core partials (the final all-reduce + mean).

Engine layout per Gram tile: D=128 features on the SBUF partition axis, so
every tile is one K=128 matmul; ScalarE applies exp(2*x) on 2048-wide PSUM
spans (4 banks) with accum_out producing row-sum partials.
"""

import os

import numpy as np

N = 16384
D = 128
NCORES = 8
B = N // NCORES  # 2048 rows per core
TAU = 0.5
SCALE = 1.0 / TAU
EPS = 1e-12

M_CH = B // 128  # 16 row chunks of 128

# phase A/B (E11, E22): uniform 2048-wide spans
SPAN_A = 2048
NSP_A = N // SPAN_A  # 8
# phase C (E12): 1024-wide spans; PSUM = 3 sim slots (6 banks) + 2 cs banks
SPAN_C = 1024
NSP_C = N // SPAN_C  # 16

_cache = {}


def _build():
    from contextlib import ExitStack

    import concourse.bass as bass
    import concourse.mybir as mybir
    from concourse import bacc
    from concourse.tile import TileContext

    f32 = mybir.dt.float32
    bf16 = mybir.dt.bfloat16
    Exp = mybir.ActivationFunctionType.Exp
    add = mybir.AluOpType.add
    AX = mybir.AxisListType.X

    # Bacc (vs plain Bass) runs the wait-legalization passes at finalize:
    # move_matmul_waits_to_ldweights + generate_event_semaphores (TRN2 allows
    # at most one sync wait per hardware instruction).
    nc = bacc.Bacc(None, target_bir_lowering=False, name="contrastive_loss")

    z1t = nc.declare_dram_parameter("z1t", [D, N], bf16, isOutput=False)
    z2t = nc.declare_dram_parameter("z2t", [D, N], bf16, isOutput=False)
    zb = nc.declare_dram_parameter("zb", [D, 2 * B], bf16, isOutput=False)

    rs11_d = nc.declare_dram_parameter("rs11", [128, M_CH], f32, isOutput=True)
    rs12_d = nc.declare_dram_parameter("rs12", [128, M_CH], f32, isOutput=True)
    rs22_d = nc.declare_dram_parameter("rs22", [128, M_CH], f32, isOutput=True)
    # E12 column-sum partials: slot j = columns [512*j, 512*(j+1))
    cs_d = nc.declare_dram_parameter("cs", [1, N // 512, 512], f32, isOutput=True)
    # raw dot-product diagonals: [d11 | d12 | d22], each B long
    dg_d = nc.declare_dram_parameter("diags", [1, 3 * B], f32, isOutput=True)

    with TileContext(nc) as tc, ExitStack() as ctx:
        const = ctx.enter_context(tc.tile_pool(name="const", bufs=1))
        work = ctx.enter_context(tc.tile_pool(name="work", bufs=4))
        prodp = ctx.enter_context(tc.tile_pool(name="prodp", bufs=3))
        outp = ctx.enter_context(tc.tile_pool(name="outp", bufs=1))
        esbp = ctx.enter_context(tc.tile_pool(name="esbp", bufs=2))

        z1t_sb = const.tile([128, N], bf16)
        z2t_sb = const.tile([128, N], bf16)
        zb_sb = const.tile([128, 2 * B], bf16)
        nc.sync.dma_start(out=zb_sb, in_=zb[:, :])
        ncol = N // 4
        for i in range(4):
            nc.sync.dma_start(
                out=z1t_sb[:, i * ncol : (i + 1) * ncol],
                in_=z1t[:, i * ncol : (i + 1) * ncol],
            )
        for i in range(4):
            nc.sync.dma_start(
                out=z2t_sb[:, i * ncol : (i + 1) * ncol],
                in_=z2t[:, i * ncol : (i + 1) * ncol],
            )
        z1b_sb = zb_sb[:, 0:B]
        z2b_sb = zb_sb[:, B : 2 * B]

        ones_f = const.tile([128, 1], f32)
        nc.vector.memset(ones_f, 1.0)
        ones_b = const.tile([128, 1], bf16)
        nc.vector.memset(ones_b, 1.0)

        dg_sb = outp.tile([1, 3 * B], f32)
        rs11_sb = outp.tile([128, M_CH], f32, tag="rs11")
        rs12_sb = outp.tile([128, M_CH], f32, tag="rs12")
        rs22_sb = outp.tile([128, M_CH], f32, tag="rs22")
        cs_sb = outp.tile([1, N // 512, 512], f32, tag="cs")
        parts12 = outp.tile([128, NSP_C * M_CH], f32, tag="p12")

        # ---- Phase 0 + E11 + E22: uniform 2048-spans, 2x4-bank ping-pong ----
        with tc.tile_pool(name="psA", bufs=2, space="PSUM") as psA:
            # Phase 0: raw diagonals diag[i] = sum_d a[d,i]*b[d,i] via
            # elementwise product + ones-matmul column sum.
            pairs = [(z1b_sb, z1b_sb), (z1b_sb, z2b_sb), (z2b_sb, z2b_sb)]
            for di, (a, b) in enumerate(pairs):
                prod = prodp.tile([128, B], f32)
                nc.vector.tensor_mul(prod, a, b)
                ps = psA.tile([128, SPAN_A], f32, tag="sim")
                for k in range(4):
                    nc.tensor.matmul(
                        ps[0:1, k * 512 : (k + 1) * 512],
                        lhsT=ones_f,
                        rhs=prod[:, k * 512 : (k + 1) * 512],
                        start=True,
                        stop=True,
                    )
                nc.vector.tensor_copy(
                    out=dg_sb[0:1, di * B : (di + 1) * B], in_=ps[0:1, 0:B]
                )
            nc.sync.dma_start(out=dg_d[:, :], in_=dg_sb)

            # E11 / E22: exp-Gram row sums, m-outer (stationary weights)
            for blk, full, rs_sb, rs_d in (
                (z1b_sb, z1t_sb, rs11_sb, rs11_d),
                (z2b_sb, z2t_sb, rs22_sb, rs22_d),
            ):
                for m in range(M_CH):
                    parts = work.tile([128, NSP_A], f32, tag="parts")
                    for s in range(NSP_A):
                        sim = psA.tile([128, SPAN_A], f32, tag="sim")
                        for k in range(4):
                            col = s * SPAN_A + k * 512
                            nc.tensor.matmul(
                                sim[:, k * 512 : (k + 1) * 512],
                                lhsT=blk[:, m * 128 : (m + 1) * 128],
                                rhs=full[:, col : col + 512],
                                start=True,
                                stop=True,
                            )
                        nc.scalar.activation(
                            out=sim,
                            in_=sim,
                            func=Exp,
                            scale=SCALE,
                            accum_out=parts[:, s : s + 1],
                        )
                    nc.vector.tensor_reduce(
                        out=rs_sb[:, m : m + 1], in_=parts, axis=AX, op=add
                    )
                nc.sync.dma_start(out=rs_d[:, :], in_=rs_sb)

        # ---- E12: s-outer, m-inner; exp once -> E_sb (bf16); row sums via
        # accum_out, column sums via ones-matmuls, each 512-chunk accumulator
        # owning its own PSUM bank at partition 0 ----
        with (
            tc.tile_pool(name="psC", bufs=3, space="PSUM") as psC,
            tc.tile_pool(name="psCS", bufs=2, space="PSUM") as psCS,
        ):
            for s in range(NSP_C):
                cs_ps = [
                    psCS.tile([1, 512], f32, tag="cs", name=f"cs_ps_{s}_{k}")
                    for k in range(2)
                ]
                for m in range(M_CH):
                    sim = psC.tile([128, SPAN_C], f32, tag="sim")
                    for k in range(2):
                        col = s * SPAN_C + k * 512
                        nc.tensor.matmul(
                            sim[:, k * 512 : (k + 1) * 512],
                            lhsT=z1b_sb[:, m * 128 : (m + 1) * 128],
                            rhs=z2t_sb[:, col : col + 512],
                            start=True,
                            stop=True,
                        )
                    esb = esbp.tile([128, SPAN_C], bf16, tag="esb")
                    nc.scalar.activation(
                        out=esb,
                        in_=sim,
                        func=Exp,
                        scale=SCALE,
                        accum_out=parts12[:, s * M_CH + m : s * M_CH + m + 1],
                    )
                    for k in range(2):
                        nc.tensor.matmul(
                            cs_ps[k][0:1, :],
                            lhsT=ones_b,
                            rhs=esb[:, k * 512 : (k + 1) * 512],
                            start=(m == 0),
                            stop=(m == M_CH - 1),
                        )
                for k in range(2):
                    nc.vector.tensor_copy(
                        out=cs_sb[0:1, 2 * s + k, :], in_=cs_ps[k][0:1, :]
                    )
            # rs12[m] = sum_s parts12[s, m]
            nc.vector.tensor_reduce(
                out=rs12_sb,
                in_=parts12.rearrange("p (s m) -> p m s", s=NSP_C),
                axis=AX,
                op=add,
            )
            nc.sync.dma_start(out=rs12_d[:, :], in_=rs12_sb)
            nc.sync.dma_start(out=cs_d[:, :, :], in_=cs_sb)

    nc.finalize()  # Bacc: runs wait-legalization + register allocation
    return nc


def _get_nc():
    if "nc" not in _cache:
        _cache["nc"] = _build()
    return _cache["nc"]


def kernel(z1: np.ndarray, z2: np.ndarray) -> np.ndarray:
    import ml_dtypes

    from concourse.bass_utils import run_bass_kernel_spmd

    z1 = np.asarray(z1, dtype=np.float32)
    z2 = np.asarray(z2, dtype=np.float32)

    # host: L2 row-normalize (matches F.normalize eps clamp), transpose to
    # feature-major, cast bf16
    def prep(z):
        n = np.sqrt((z.astype(np.float64) ** 2).sum(axis=1, keepdims=True))
        zn = (z / np.maximum(n, EPS).astype(np.float32)).astype(np.float32)
        return np.ascontiguousarray(zn.T).astype(ml_dtypes.bfloat16)

    z1tn = prep(z1)  # [D, N] bf16
    z2tn = prep(z2)

    core_ids = list(range(NCORES))
    in_maps = []
    for c in core_ids:
        sl = slice(c * B, (c + 1) * B)
        in_maps.append(
            {
                "z1t": z1tn,
                "z2t": z2tn,
                "zb": np.ascontiguousarray(
                    np.concatenate([z1tn[:, sl], z2tn[:, sl]], axis=1)
                ),
            }
        )

    nc = _get_nc()
    res = run_bass_kernel_spmd(
        nc,
        in_maps,
        core_ids,
        trace=bool(int(os.environ.get("KERNEL_TRACE", "0"))),
    )
    _cache["last_result"] = res

    # ---- host combine (the final all-reduce / mean) ----
    # global column sums of exp(2*S12): sum partials over cores
    cs_global = np.zeros(N, dtype=np.float64)
    for c in core_ids:
        cs_global += res.results[c]["cs"].astype(np.float64).reshape(N)

    loss_sum = 0.0
    for c in core_ids:
        r = res.results[c]
        rs11 = r["rs11"].astype(np.float64).T.reshape(B)
        rs12 = r["rs12"].astype(np.float64).T.reshape(B)
        rs22 = r["rs22"].astype(np.float64).T.reshape(B)
        dg = r["diags"].astype(np.float64).reshape(3 * B)
        d11, d12, d22 = dg[0:B], dg[B : 2 * B], dg[2 * B : 3 * B]
        rs21 = cs_global[c * B : (c + 1) * B]
        den1 = rs11 - np.exp(SCALE * d11) + rs12
        den2 = rs22 - np.exp(SCALE * d22) + rs21
        l = 0.5 * (np.log(den1) + np.log(den2)) - SCALE * d12
        loss_sum += l.sum()

    return np.float32(loss_sum / N)


# revision 21
# speedup vs baseline: 1.6552x; 1.3037x over previous
"""Contrastive loss (SimCLR-style semi_loss pair) on 8 Trainium2 NeuronCores.

Math (reference):
    z1n, z2n = L2-normalized rows of z1, z2            # [N, D], N=16384, D=128
    S11 = z1n @ z1n.T, S12 = z1n @ z2n.T, S22 = z2n @ z2n.T, S21 = S12.T
    d1_i = sum_j exp(2*S11_ij) - exp(2*S11_ii) + sum_j exp(2*S12_ij)
    d2_i = sum_j exp(2*S22_ij) - exp(2*S22_ii) + sum_j exp(2*S21_ij)
    loss = mean_i( 0.5*(log d1_i + log d2_i) - 2*S12_ii )

Device strategy (row-parallel over N, 8 cores, D=128 on the partition axis
so every Gram tile is one K=128 matmul):

* exp(2*S12) values are computed ONCE per element (ScalarE), written to
  SBUF as bf16; row sums come from the activation accumulator, column sums
  (= exp(2*S21) row sums) from ones-vector matmuls accumulated in PSUM.
* S11/S22 are symmetric: only block-column spans s >= g are exponentiated
  (g = row-chunk group, 1024-row granularity). The lower-triangle part of
  each row sum is recovered from column sums of the computed upper part
  (diagonal 1024x1024 block squares are computed by both sides and
  excluded from the column sums to avoid double counting).
* SPMD: one NEFF for all cores. Refl streams need every core to see the
  same column-range structure, so rows are sharded STRIDED: core c owns
  row chunks {128*(8g+c)} for g=0..15; chunk g covers columns
  [1024g, 16384). Host packs the per-core row blocks into the `zb` input
  and maps everything back when combining.
* Host does the final O(N) combine: all-reduce of column-sum partials,
  logs, mean.
"""

import os

import numpy as np

N = 16384
D = 128
NCORES = 8
B = N // NCORES  # 2048 rows per core
TAU = 0.5
SCALE = 1.0 / TAU
EPS = 1e-12

G = 16  # row-chunk groups per core (128 rows each); chunk g -> cols >= 1024g
SPAN = 1024
NSP = N // SPAN  # 16 spans

_cache = {}


def _build():
    from contextlib import ExitStack

    import concourse.mybir as mybir
    from concourse import bacc
    from concourse.tile import TileContext

    f32 = mybir.dt.float32
    bf16 = mybir.dt.bfloat16
    Exp = mybir.ActivationFunctionType.Exp
    add = mybir.AluOpType.add
    AX = mybir.AxisListType.X

    # Bacc (vs plain Bass) runs the wait-legalization passes at finalize:
    # move_matmul_waits_to_ldweights + generate_event_semaphores (TRN2 allows
    # at most one sync wait per hardware instruction).
    nc = bacc.Bacc(None, target_bir_lowering=False, name="contrastive_loss")

    z1t = nc.declare_dram_parameter("z1t", [D, N], bf16, isOutput=False)
    z2t = nc.declare_dram_parameter("z2t", [D, N], bf16, isOutput=False)
    # per-core strided row chunks: [z1 chunks g=0..15 | z2 chunks g=0..15]
    zb = nc.declare_dram_parameter("zb", [D, 2 * B], bf16, isOutput=False)

    rs11_d = nc.declare_dram_parameter("rs11", [128, G], f32, isOutput=True)
    rs22_d = nc.declare_dram_parameter("rs22", [128, G], f32, isOutput=True)
    rs12_d = nc.declare_dram_parameter("rs12", [128, G], f32, isOutput=True)
    # column-sum partials, 512-wide chunks: cs11/cs22 spans 1..15 (30 chunks),
    # cs12 spans 0..15 (32 chunks)
    cs11_d = nc.declare_dram_parameter("cs11", [1, 30, 512], f32, isOutput=True)
    cs22_d = nc.declare_dram_parameter("cs22", [1, 30, 512], f32, isOutput=True)
    cs12_d = nc.declare_dram_parameter("cs12", [1, 32, 512], f32, isOutput=True)
    # raw diagonal dot products: pairs (z1,z1),(z1,z2),(z2,z2), 4 chunks each
    dg_d = nc.declare_dram_parameter("diags", [1, 12, 512], f32, isOutput=True)

    with TileContext(nc) as tc, ExitStack() as ctx:
        const = ctx.enter_context(tc.tile_pool(name="const", bufs=1))
        prodp = ctx.enter_context(tc.tile_pool(name="prodp", bufs=3))
        outp = ctx.enter_context(tc.tile_pool(name="outp", bufs=1))
        esbp = ctx.enter_context(tc.tile_pool(name="esbp", bufs=3))
        stag = ctx.enter_context(tc.tile_pool(name="stag", bufs=6))
        psC = ctx.enter_context(tc.tile_pool(name="psC", bufs=3, space="PSUM"))
        psCS = ctx.enter_context(tc.tile_pool(name="psCS", bufs=2, space="PSUM"))

        z1t_sb = const.tile([128, N], bf16)
        z2t_sb = const.tile([128, N], bf16)
        zb_sb = const.tile([128, 2 * B], bf16)
        nc.sync.dma_start(out=zb_sb, in_=zb[:, :])
        ncol = N // 4
        for i in range(4):
            nc.sync.dma_start(
                out=z1t_sb[:, i * ncol : (i + 1) * ncol],
                in_=z1t[:, i * ncol : (i + 1) * ncol],
            )
        for i in range(4):
            nc.sync.dma_start(
                out=z2t_sb[:, i * ncol : (i + 1) * ncol],
                in_=z2t[:, i * ncol : (i + 1) * ncol],
            )
        z1b_sb = zb_sb[:, 0:B]
        z2b_sb = zb_sb[:, B : 2 * B]

        ones_f = const.tile([128, 1], f32)
        nc.vector.memset(ones_f, 1.0)
        ones_b = const.tile([128, 1], bf16)
        nc.vector.memset(ones_b, 1.0)

        rs11_sb = outp.tile([128, G], f32, tag="rs11")
        rs22_sb = outp.tile([128, G], f32, tag="rs22")
        rs12_sb = outp.tile([128, G], f32, tag="rs12")
        parts11 = outp.tile([128, G * NSP], f32, tag="p11")
        parts22 = outp.tile([128, G * NSP], f32, tag="p22")
        parts12 = outp.tile([128, G * NSP], f32, tag="p12")

        def evac(cs_ps, dram_slot, name):
            """PSUM [1,512] accumulator -> SBUF staging -> DRAM."""
            st = stag.tile([1, 512], f32, tag="stage", name=name)
            nc.vector.tensor_copy(out=st, in_=cs_ps[0:1, :])
            nc.sync.dma_start(out=dram_slot, in_=st)

        # ---- Phase 0: raw diagonals diag[i] = sum_d a[d,i]*b[d,i] ----
        pairs = [(z1b_sb, z1b_sb), (z1b_sb, z2b_sb), (z2b_sb, z2b_sb)]
        for di, (a, b) in enumerate(pairs):
            prod = prodp.tile([128, B], f32)
            nc.vector.tensor_mul(prod, a, b)
            for k in range(4):
                dps = psCS.tile([1, 512], f32, tag="cs", name=f"dg_{di}_{k}")
                nc.tensor.matmul(
                    dps[0:1, :],
                    lhsT=ones_f,
                    rhs=prod[:, k * 512 : (k + 1) * 512],
                    start=True,
                    stop=True,
                )
                evac(dps, dg_d[0:1, 4 * di + k, :], f"dgs_{di}_{k}")

        # ---- refl streams E11/E22: block-upper-triangle only ----
        # chunk g (weights zb[:, off + 128g]) x span s >= g; rowsum partials
        # via ACT accumulator; column sums over g < s (diag square excluded)
        for zoff, full, parts, cs_d_ in (
            (0, z1t_sb, parts11, cs11_d),
            (B, z2t_sb, parts22, cs22_d),
        ):
            for s in range(NSP):
                if s > 0:
                    cs_ps = [
                        psCS.tile([1, 512], f32, tag="cs", name=f"cs_{zoff}_{s}_{k}")
                        for k in range(2)
                    ]
                for g in range(s + 1):
                    sim = psC.tile([128, SPAN], f32, tag="sim", name="sim_t")
                    for k in range(2):
                        col = s * SPAN + k * 512
                        nc.tensor.matmul(
                            sim[:, k * 512 : (k + 1) * 512],
                            lhsT=zb_sb[:, zoff + g * 128 : zoff + (g + 1) * 128],
                            rhs=full[:, col : col + 512],
                            start=True,
                            stop=True,
                        )
                    esb = esbp.tile([128, SPAN], bf16, tag="esb", name="esb_t")
                    nc.scalar.activation(
                        out=esb,
                        in_=sim,
                        func=Exp,
                        scale=SCALE,
                        accum_out=parts[:, g * NSP + s : g * NSP + s + 1],
                    )
                    if g < s:
                        for k in range(2):
                            nc.tensor.matmul(
                                cs_ps[k][0:1, :],
                                lhsT=ones_b,
                                rhs=esb[:, k * 512 : (k + 1) * 512],
                                start=(g == 0),
                                stop=(g == s - 1),
                            )
                if s > 0:
                    for k in range(2):
                        evac(
                            cs_ps[k],
                            cs_d_[0:1, 2 * (s - 1) + k, :],
                            f"css_{zoff}_{s}_{k}",
                        )

        # ---- E12: full rectangle; exp once, rowsums + column sums ----
        for s in range(NSP):
            cs_ps = [
                psCS.tile([1, 512], f32, tag="cs", name=f"cs12_{s}_{k}")
                for k in range(2)
            ]
            for m in range(G):
                sim = psC.tile([128, SPAN], f32, tag="sim", name="sim_r")
                for k in range(2):
                    col = s * SPAN + k * 512
                    nc.tensor.matmul(
                        sim[:, k * 512 : (k + 1) * 512],
                        lhsT=zb_sb[:, m * 128 : (m + 1) * 128],
                        rhs=z2t_sb[:, col : col + 512],
                        start=True,
                        stop=True,
                    )
                esb = esbp.tile([128, SPAN], bf16, tag="esb", name="esb_r")
                nc.scalar.activation(
                    out=esb,
                    in_=sim,
                    func=Exp,
                    scale=SCALE,
                    accum_out=parts12[:, m * NSP + s : m * NSP + s + 1],
                )
                for k in range(2):
                    nc.tensor.matmul(
                        cs_ps[k][0:1, :],
                        lhsT=ones_b,
                        rhs=esb[:, k * 512 : (k + 1) * 512],
                        start=(m == 0),
                        stop=(m == G - 1),
                    )
            for k in range(2):
                evac(cs_ps[k], cs12_d[0:1, 2 * s + k, :], f"cs12s_{s}_{k}")

        # ---- final row-sum reductions ----
        for g in range(G):
            nc.vector.tensor_reduce(
                out=rs11_sb[:, g : g + 1],
                in_=parts11[:, g * NSP + g : (g + 1) * NSP],
                axis=AX,
                op=add,
            )
            nc.vector.tensor_reduce(
                out=rs22_sb[:, g : g + 1],
                in_=parts22[:, g * NSP + g : (g + 1) * NSP],
                axis=AX,
                op=add,
            )
        nc.vector.tensor_reduce(
            out=rs12_sb,
            in_=parts12.rearrange("p (m s) -> p m s", m=G),
            axis=AX,
            op=add,
        )
        nc.sync.dma_start(out=rs11_d[:, :], in_=rs11_sb)
        nc.sync.dma_start(out=rs22_d[:, :], in_=rs22_sb)
        nc.sync.dma_start(out=rs12_d[:, :], in_=rs12_sb)

    nc.finalize()  # Bacc: runs wait-legalization + register allocation
    return nc


def _get_nc():
    if "nc" not in _cache:
        _cache["nc"] = _build()
    return _cache["nc"]


def kernel(z1: np.ndarray, z2: np.ndarray) -> np.ndarray:
    import ml_dtypes

    from concourse.bass_utils import run_bass_kernel_spmd

    z1 = np.asarray(z1, dtype=np.float32)
    z2 = np.asarray(z2, dtype=np.float32)

    # host: L2 row-normalize (matches F.normalize eps clamp), transpose to
    # feature-major, cast bf16
    def prep(z):
        n = np.sqrt((z.astype(np.float64) ** 2).sum(axis=1, keepdims=True))
        zn = (z / np.maximum(n, EPS).astype(np.float32)).astype(np.float32)
        return np.ascontiguousarray(zn.T).astype(ml_dtypes.bfloat16)

    z1tn = prep(z1)  # [D, N] bf16
    z2tn = prep(z2)

    core_ids = list(range(NCORES))
    # strided row chunks: core c, group g -> rows [128*(8g+c), +128)
    in_maps = []
    for c in core_ids:
        cols = np.concatenate(
            [np.arange(128 * (8 * g + c), 128 * (8 * g + c) + 128) for g in range(G)]
        )
        in_maps.append(
            {
                "z1t": z1tn,
                "z2t": z2tn,
                "zb": np.ascontiguousarray(
                    np.concatenate([z1tn[:, cols], z2tn[:, cols]], axis=1)
                ),
            }
        )

    nc = _get_nc()
    res = run_bass_kernel_spmd(
        nc,
        in_maps,
        core_ids,
        trace=bool(int(os.environ.get("KERNEL_TRACE", "0"))),
    )
    _cache["last_result"] = res

    # ---- host combine (the final all-reduce / mean) ----
    # global column-sum vectors (sum partials over cores)
    def gather_cs(name, first_span):
        v = np.zeros(N, dtype=np.float64)
        for c in core_ids:
            arr = res.results[c][name].astype(np.float64).reshape(-1, 512)
            v[first_span * SPAN :] += arr.reshape(-1)
        return v

    cs11_g = gather_cs("cs11", 1)
    cs22_g = gather_cs("cs22", 1)
    cs12_g = gather_cs("cs12", 0)

    loss_sum = 0.0
    for c in core_ids:
        r = res.results[c]
        # local index l = g*128 + p  ->  global row 128*(8g+c) + p
        gl = np.concatenate(
            [np.arange(128 * (8 * g + c), 128 * (8 * g + c) + 128) for g in range(G)]
        )
        rs11 = r["rs11"].astype(np.float64).T.reshape(B)
        rs22 = r["rs22"].astype(np.float64).T.reshape(B)
        rs12 = r["rs12"].astype(np.float64).T.reshape(B)
        dg = r["diags"].astype(np.float64).reshape(3, B)
        d11, d12, d22 = dg[0], dg[1], dg[2]
        den1 = rs11 + cs11_g[gl] - np.exp(SCALE * d11) + rs12
        den2 = rs22 + cs22_g[gl] - np.exp(SCALE * d22) + cs12_g[gl]
        l = 0.5 * (np.log(den1) + np.log(den2)) - SCALE * d12
        loss_sum += l.sum()

    return np.float32(loss_sum / N)
